# revision 2
# baseline (speedup 1.0000x reference)
"""Trainium2 Bass kernel for LowRankRayTracer.

csi[f] = (delta_t/D) * v_f^T M v_f,  M = conj(rad)^T conj(att)  (R=32, complex)
contracted over N = D*K = 524288 rows.

Strategy (8 cores):
  - Shard the N rows across cores (512 directions each). csi is linear in M,
    so each core computes its partial S = rad32^T att32 (64x64, f32 view of
    complex pairs -> all four real cross products at once), builds
    W = [W_real | W_imag] (block form), computes partial csi over ALL F=8192
    subcarriers, and the host just sums the 8 partial csi vectors.
  - fp32 matmul is 4 cyc/col on TRN2 PE, so inputs are split on the host into
    fp16 hi+lo (same total bytes); with the att hi/lo pair packed side by side
    as one 256-wide moving operand, two matmuls per slice (lhsT=rad_h, rad_l)
    produce all four products hh|hl|lh|ll -- exact reconstruction, and half
    the LDWEIGHTS of a 3-pass version (LDW is the PE bottleneck otherwise).
  - Matmuls accumulate round-robin into 4 PSUM banks (avoids same-bank RMW
    serialization); diagonal blocks summed later via selection matmuls.
"""

import numpy as np

D, K, R = 4096, 128, 32
F = 8192
N_CORES = 8
DIR_PER_CORE = D // N_CORES              # 512
ROWS_PER_CORE = DIR_PER_CORE * K         # 65536 rows of (64,) f32
N_MACRO = 8                              # macro tiles per tensor per core
MACRO_COLS = 4096                        # fp16 per partition per macro tile
SLICE = 128                              # matmul slice width (2 rows/partition)
SCALE = (200.0 / K) / D                  # delta_t / num_directions (exact binary)
FCHUNK = 512                             # phase-3 subcarriers per chunk
N_FCHUNK = F // FCHUNK                   # 16
NB = 4                                   # round-robin PSUM accumulator banks

_NC_CACHE = {}


def _build_consts():
    """(128, 258) f32: four (128,64) selection matrices + ones-selector cols."""
    c = np.zeros((128, 258), np.float32)
    EA = np.zeros((128, 32), np.float32)
    OA = np.zeros((128, 32), np.float32)
    EB = np.zeros((128, 32), np.float32)
    OB = np.zeros((128, 32), np.float32)
    for m in range(32):
        EA[2 * m, m] = 1.0
        OA[2 * m + 1, m] = 1.0
        EB[64 + 2 * m, m] = 1.0
        OB[64 + 2 * m + 1, m] = 1.0
    c[:, 0:32] = EA
    c[:, 32:64] = OA
    c[:, 64:96] = EB
    c[:, 96:128] = OB
    c[:, 128:160] = OA
    c[:, 160:192] = EA
    c[:, 192:224] = OB
    c[:, 224:256] = EB
    c[0:64, 256] = 1.0
    c[64:128, 257] = 1.0
    return c


def build_nc(n_macro=N_MACRO):
    import concourse.bacc as bacc
    import concourse.mybir as mybir
    import concourse.tile as tile

    fp32 = mybir.dt.float32
    fp16 = mybir.dt.float16
    nc = bacc.Bacc(trn_type="TRN2", target_bir_lowering=False, debug=False)

    rad_h_d = nc.dram_tensor("rad_h", [n_macro, 128, MACRO_COLS], fp16,
                             kind="ExternalInput").ap()
    rad_l_d = nc.dram_tensor("rad_l", [n_macro, 128, MACRO_COLS], fp16,
                             kind="ExternalInput").ap()
    att_hl_d = nc.dram_tensor("att_hl", [n_macro, 128, 2 * MACRO_COLS], fp16,
                              kind="ExternalInput").ap()
    gtd_d = nc.dram_tensor("gtd", [128, F], fp32, kind="ExternalInput").ap()
    gth_d = nc.dram_tensor("gth", [64, F], fp16, kind="ExternalInput").ap()
    gtl_d = nc.dram_tensor("gtl", [64, F], fp16, kind="ExternalInput").ap()
    cst_d = nc.dram_tensor("consts", [128, 258], fp32, kind="ExternalInput").ap()
    out_d = nc.dram_tensor("csi", [2, F], fp32, kind="ExternalOutput").ap()

    with tile.TileContext(nc) as tc:
        with (
            tc.tile_pool(name="io", bufs=2) as io_pool,
            tc.tile_pool(name="small", bufs=1) as small,
            tc.tile_pool(name="epool", bufs=8) as epool,
        ):
            # constants up front (tiny); gtd issued after the main-loop DMAs
            # so it doesn't steal early HBM bandwidth (not needed till phase 3)
            c_sb = small.tile([128, 258], fp32, tag="consts")
            nc.sync.dma_start(c_sb[:], cst_d[:])
            gtd_sb = small.tile([128, F], fp32, tag="gtd")
            gth_sb = small.tile([64, F], fp16, tag="gth")
            gtl_sb = small.tile([64, F], fp16, tag="gtl")

            # ---- main loop: S += rad^T att via fp16 hi/lo, 256-wide rhs ----
            # lhsT=rad_h over rhs=[att_h|att_l] gives [hh|hl]; lhsT=rad_l
            # gives [lh|ll]. S = sum of all four 128-col blocks (exact).
            s_sb = small.tile([128, 128], fp32, tag="s_sb")
            n_slices = MACRO_COLS // SLICE
            total = n_macro * n_slices * 2
            with tc.tile_pool(name="spsum", bufs=1, space="PSUM") as spsum:
                banks = [spsum.tile([128, 2 * SLICE], fp32, tag=f"s{b}",
                                    name=f"sbank{b}")
                         for b in range(NB)]
                seen = [False] * NB
                idx = 0
                for i in range(n_macro):
                    rad_h = io_pool.tile([128, MACRO_COLS], fp16, tag="rad_h")
                    rad_l = io_pool.tile([128, MACRO_COLS], fp16, tag="rad_l")
                    att_hl = io_pool.tile([128, 2 * MACRO_COLS], fp16,
                                          tag="att_hl")
                    if i == 0:
                        # halve the first loads so the first matmuls start
                        # as soon as ~1.5 MiB has landed, not 4 MiB
                        hm = MACRO_COLS // 2
                        nc.sync.dma_start(rad_h[:, 0:hm], rad_h_d[0, :, 0:hm])
                        nc.scalar.dma_start(att_hl[:, 0:2 * hm],
                                            att_hl_d[0, :, 0:2 * hm])
                        nc.sync.dma_start(rad_l[:, 0:hm], rad_l_d[0, :, 0:hm])
                        nc.sync.dma_start(rad_h[:, hm:], rad_h_d[0, :, hm:])
                        nc.scalar.dma_start(att_hl[:, 2 * hm:],
                                            att_hl_d[0, :, 2 * hm:])
                        nc.sync.dma_start(rad_l[:, hm:], rad_l_d[0, :, hm:])
                    else:
                        nc.sync.dma_start(rad_h[:], rad_h_d[i, :, :])
                        nc.sync.dma_start(rad_l[:], rad_l_d[i, :, :])
                        nc.scalar.dma_start(att_hl[:], att_hl_d[i, :, :])
                    for s in range(n_slices):
                        rsl = slice(s * SLICE, (s + 1) * SLICE)
                        asl = slice(s * 2 * SLICE, (s + 1) * 2 * SLICE)
                        for lh in (rad_h, rad_l):
                            b = idx % NB
                            nc.tensor.matmul(
                                banks[b][:],
                                lhsT=lh[:, rsl],
                                rhs=att_hl[:, asl],
                                start=not seen[b],
                                stop=(idx >= total - NB),
                            )
                            seen[b] = True
                            idx += 1

                nc.sync.dma_start(gtd_sb[:], gtd_d[:])
                nc.sync.dma_start(gth_sb[:], gth_d[:])
                nc.sync.dma_start(gtl_sb[:], gtl_d[:])

                # S = sum of all four 128-col blocks over the 4 banks
                acc = small.tile([128, 2 * SLICE], fp32, tag="acc")
                nc.vector.tensor_copy(acc[:], banks[0][:])
                for b in range(1, NB):
                    nc.vector.tensor_add(acc[:], acc[:], banks[b][:])
                nc.vector.tensor_add(s_sb[:], acc[:, 0:SLICE],
                                     acc[:, SLICE:2 * SLICE])

            # ---- epilogue: build W = [W_real | W_imag] (64, 128) ----
            with tc.tile_pool(name="vpsum", bufs=1, space="PSUM") as vpsum:
                v1 = vpsum.tile([64, 64], fp32, tag="v1")
                nc.tensor.matmul(v1[:], lhsT=c_sb[:, 0:64], rhs=s_sb[:, 0:64],
                                 start=True, stop=False)
                nc.tensor.matmul(v1[:], lhsT=c_sb[:, 64:128],
                                 rhs=s_sb[:, 64:128], start=False, stop=True)
                v2 = vpsum.tile([64, 64], fp32, tag="v2")
                nc.tensor.matmul(v2[:], lhsT=c_sb[:, 128:192],
                                 rhs=s_sb[:, 0:64], start=True, stop=False)
                nc.tensor.matmul(v2[:], lhsT=c_sb[:, 192:256],
                                 rhs=s_sb[:, 64:128], start=False, stop=True)

                v1s = small.tile([64, 64], fp32, tag="v1s")
                nc.vector.tensor_copy(v1s[:], v1[:])
                v2s = small.tile([64, 64], fp32, tag="v2s")
                nc.vector.tensor_copy(v2s[:], v2[:])

            # mr = Mr (dup-stacked), mp = -Mi (dup-stacked)
            mr = small.tile([64, 32], fp32, tag="mr")
            mp = small.tile([64, 32], fp32, tag="mp")
            nc.vector.tensor_sub(mr[0:32, :], v1s[0:32, 0:64:2], v2s[0:32, 1:64:2])
            nc.vector.tensor_sub(mr[32:64, :], v2s[32:64, 0:64:2], v1s[32:64, 1:64:2])
            nc.vector.tensor_add(mp[0:32, :], v1s[0:32, 1:64:2], v2s[0:32, 0:64:2])
            nc.vector.tensor_add(mp[32:64, :], v2s[32:64, 1:64:2], v1s[32:64, 0:64:2])

            wri = small.tile([64, 128], fp32, tag="wri")
            s_ = float(SCALE)
            # W_real = [[Mr, -Mi], [-Mi, -Mr]] * s
            nc.scalar.mul(wri[0:32, 0:32], mr[0:32, :], s_)
            nc.scalar.mul(wri[0:32, 32:64], mp[0:32, :], s_)
            nc.scalar.mul(wri[32:64, 0:32], mp[32:64, :], s_)
            nc.scalar.mul(wri[32:64, 32:64], mr[32:64, :], -s_)
            # W_imag = [[Mi, Mr], [Mr, -Mi]] * s
            nc.scalar.mul(wri[0:32, 64:96], mp[0:32, :], -s_)
            nc.scalar.mul(wri[0:32, 96:128], mr[0:32, :], s_)
            nc.scalar.mul(wri[32:64, 64:96], mr[32:64, :], s_)
            nc.scalar.mul(wri[32:64, 96:128], mp[32:64, :], s_)

            # fp16 hi/lo split of W for the phase-3 matmuls
            wh = small.tile([64, 128], fp16, tag="wh")
            nc.vector.tensor_copy(wh[:], wri[:])
            whf = small.tile([64, 128], fp32, tag="whf")
            nc.vector.tensor_copy(whf[:], wh[:])
            wlf = small.tile([64, 128], fp32, tag="wlf")
            nc.vector.tensor_sub(wlf[:], wri[:], whf[:])
            wl = small.tile([64, 128], fp16, tag="wl")
            nc.vector.tensor_copy(wl[:], wlf[:])

            # PE warm-keepers: cheap matmuls dependent on s_sb bridge the
            # epilogue gap so HAM doesn't re-throttle before phase 3
            with tc.tile_pool(name="wpsum", bufs=1, space="PSUM") as wpsum:
                warm_ps = wpsum.tile([64, 64], fp32, tag="warm")
                for w in range(10):
                    nc.tensor.matmul(warm_ps[:], lhsT=c_sb[:, 0:64],
                                     rhs=s_sb[:, 0:64], start=True, stop=True)

            # ---- phase 3: csi chunks over F ----
            # All T matmuls issued first so the per-chunk csi matmuls don't
            # head-of-line-block them in the in-order PE queue.
            csi_sb = small.tile([2, F], fp32, tag="csi_sb")
            with (
                tc.tile_pool(name="tpsum", bufs=6, space="PSUM") as tpsum,
                tc.tile_pool(name="cpsum", bufs=2, space="PSUM") as cpsum,
            ):
                t_tiles = []
                e_tiles = []
                for ci in range(N_FCHUNK):
                    fs = slice(ci * FCHUNK, (ci + 1) * FCHUNK)
                    t_ps = tpsum.tile([128, FCHUNK], fp32, tag="t",
                                      name=f"t{ci}")
                    # T = W^T g via fp16 hi/lo (dropped Wl*gl ~ 2^-22)
                    nc.tensor.matmul(t_ps[:], lhsT=wh[:], rhs=gth_sb[:, fs],
                                     start=True, stop=False)
                    nc.tensor.matmul(t_ps[:], lhsT=wl[:], rhs=gth_sb[:, fs],
                                     start=False, stop=False)
                    nc.tensor.matmul(t_ps[:], lhsT=wh[:], rhs=gtl_sb[:, fs],
                                     start=False, stop=True)
                    t_tiles.append(t_ps)
                    e_sb = epool.tile([128, FCHUNK], fp32, tag="e",
                                      name=f"e{ci}")
                    nc.vector.tensor_mul(e_sb[:], gtd_sb[:, fs], t_ps[:])
                    e_tiles.append(e_sb)
                for ci in range(N_FCHUNK):
                    fs = slice(ci * FCHUNK, (ci + 1) * FCHUNK)
                    c_ps = cpsum.tile([2, FCHUNK], fp32, tag="c",
                                      name=f"c{ci}")
                    nc.tensor.matmul(c_ps[:], lhsT=c_sb[:, 256:258],
                                     rhs=e_tiles[ci][:], start=True, stop=True)
                    nc.scalar.copy(csi_sb[:, fs], c_ps[:])

            nc.sync.dma_start(out_d[:], csi_sb[:])

    nc.compile()
    return nc


def _prep_shared(fbv):
    """gtd (128,F) f32 dup + fp16 hi/lo (64,F) from complex fbv (F, R)."""
    fbv32 = np.ascontiguousarray(fbv).view(np.float32).reshape(F, 2 * R)
    gbt = np.ascontiguousarray(
        np.concatenate([fbv32[:, 0::2].T, fbv32[:, 1::2].T], axis=0))
    gtd = np.ascontiguousarray(np.concatenate([gbt, gbt], axis=0))
    gth = gbt.astype(np.float16)
    gtl = (gbt - gth.astype(np.float32)).astype(np.float16)
    return gtd, gth, gtl


def _shard_hl(arr, core):
    """Core's complex64 shard -> (hi, lo) fp16 arrays (N_MACRO,128,MACRO_COLS)."""
    sh = arr[core * DIR_PER_CORE:(core + 1) * DIR_PER_CORE]
    f32 = np.ascontiguousarray(sh).view(np.float32).ravel()
    h = f32.astype(np.float16)
    lo = (f32 - h.astype(np.float32)).astype(np.float16)
    shp = (N_MACRO, 128, MACRO_COLS)
    return h.reshape(shp), lo.reshape(shp)


def _pack_hl(h, lo):
    """Interleave hi/lo at 128-col slice granularity: [...,s*256:+256] =
    [h_slice(128) | lo_slice(128)] -> (N_MACRO, 128, 2*MACRO_COLS)."""
    ns = MACRO_COLS // SLICE
    h4 = h.reshape(N_MACRO, 128, ns, SLICE)
    l4 = lo.reshape(N_MACRO, 128, ns, SLICE)
    return np.ascontiguousarray(
        np.stack([h4, l4], axis=3).reshape(N_MACRO, 128, 2 * MACRO_COLS))


def _build_in_maps(attenuation_vectors, radiation_vectors,
                   frequency_basis_vectors):
    gtd, gth, gtl = _prep_shared(frequency_basis_vectors)
    consts = _build_consts()
    in_maps = []
    for c in range(N_CORES):
        rh, rl = _shard_hl(radiation_vectors, c)
        ah, al = _shard_hl(attenuation_vectors, c)
        in_maps.append({
            "rad_h": rh, "rad_l": rl,
            "att_hl": _pack_hl(ah, al),
            "gtd": gtd, "gth": gth, "gtl": gtl,
            "consts": consts,
        })
    return in_maps


def kernel(attenuation_vectors, radiation_vectors, frequency_basis_vectors):
    from concourse.bass_utils import run_bass_kernel_spmd

    if "nc" not in _NC_CACHE:
        _NC_CACHE["nc"] = build_nc()
    nc = _NC_CACHE["nc"]

    in_maps = _build_in_maps(attenuation_vectors, radiation_vectors,
                             frequency_basis_vectors)
    res = run_bass_kernel_spmd(nc, in_maps, core_ids=list(range(N_CORES)))
    acc = np.zeros((2, F), np.float64)
    for r in res.results:
        acc += r["csi"]
    return (acc[0] + 1j * acc[1]).astype(np.complex64)



# revision 4
# speedup vs baseline: 2.2167x; 2.2167x over previous
"""Trainium2 Bass kernel for LowRankRayTracer.

csi[f] = (delta_t/D) * v_f^T M v_f,  M = conj(rad)^T conj(att)  (R=32, complex)
contracted over N = D*K = 524288 rows.

Strategy (8 cores, memory-bound => minimize HBM bytes):
  - Tolerance is 2e-2; fp16-only inputs give ~5e-4 (validated in sim), so
    each f32 component is shipped as ONE fp16 (half the baseline's hi+lo
    traffic): 16 MiB/core of ray data + 2 MiB of frequency data.
  - Rows sharded 8 ways. Each core accumulates s128 = rad_pack^T att_pack
    (128x128, 2 rows/partition-slice) in 4 round-robin PSUM banks, then
    16 tiny selection matmuls (+-1 consts) assemble W = [W_real|W_imag]
    directly from s128 (host packs [Re|Im] per row so all extractions are
    32-aligned). Phase 3: T = W^T g per 512-col chunk, e = g (.) T (DVE),
    csi = ones^T e (PE), partial csi summed on host across cores.
  - All chunk tiles are statically resident in SBUF (20 MiB used) so every
    DMA is issued up front and the stream never stalls; first/last chunks
    are split so compute starts early and the post-DMA tail is short.
"""

import numpy as np

D, K, R = 4096, 128, 32
F = 8192
N_CORES = 8
DIR_PER_CORE = D // N_CORES              # 512
N_CHUNK = 4                              # macro chunks per tensor per core
CHUNK_COLS = 8192                        # fp16 per partition per chunk
SLICES = CHUNK_COLS // 128               # 64 matmul slices per chunk
SCALE = (200.0 / K) / D                  # delta_t / num_directions
FCHUNK = 512
N_FCHUNK = F // FCHUNK                   # 16
NB = 4                                   # round-robin PSUM accumulator banks
SEL_COLS = 1152                          # 16*64 selection cols + ones2 + pad

_NC_CACHE = {}


def _build_sel():
    """(128, 1152) f16: 16 (128,64) +-1 selection matrices + ones2 cols.

    W 32-block (R,C) = sum of sigma*Q_uv;  Q_uv = S64[32u:+32, 32v:+32],
    S64 = s128[0:64,0:64] + s128[64:,64:].  Matmul g uses rhs cols
    [32g:32g+32] of s128; source (side, v) -> g = v + 2*side.
    """
    table = {
        (0, 0): [(+1, 0, 0), (-1, 1, 1)],   # Mr
        (1, 0): [(+1, 0, 1), (+1, 1, 0)],   # -Mi
        (0, 1): [(+1, 0, 1), (+1, 1, 0)],   # -Mi
        (1, 1): [(-1, 0, 0), (+1, 1, 1)],   # -Mr
        (0, 2): [(-1, 0, 1), (-1, 1, 0)],   # Mi
        (1, 2): [(+1, 0, 0), (-1, 1, 1)],   # Mr
        (0, 3): [(+1, 0, 0), (-1, 1, 1)],   # Mr
        (1, 3): [(+1, 0, 1), (+1, 1, 0)],   # -Mi
    }
    sel = np.zeros((128, SEL_COLS), np.float16)
    for (Rb, C), terms in table.items():
        for sigma, u, v in terms:
            for side in (0, 1):
                g = v + 2 * side
                for r0 in range(32):
                    p = 64 * side + 32 * u + r0
                    sel[p, (C * 4 + g) * 64 + 32 * Rb + r0] = sigma
    sel[0:64, 1024] = 1.0
    sel[64:128, 1025] = 1.0
    return sel


def build_nc():
    import concourse.bacc as bacc
    import concourse.mybir as mybir
    import concourse.tile as tile

    fp32 = mybir.dt.float32
    fp16 = mybir.dt.float16
    nc = bacc.Bacc(trn_type="TRN2", target_bir_lowering=False, debug=False)

    rad_d = nc.dram_tensor("rad", [N_CHUNK, 128, CHUNK_COLS], fp16,
                           kind="ExternalInput").ap()
    att_d = nc.dram_tensor("att", [N_CHUNK, 128, CHUNK_COLS], fp16,
                           kind="ExternalInput").ap()
    gtd_d = nc.dram_tensor("gtd", [128, F], fp16, kind="ExternalInput").ap()
    sel_d = nc.dram_tensor("sel", [128, SEL_COLS], fp16,
                           kind="ExternalInput").ap()
    out_d = nc.dram_tensor("csi", [2, F], fp16, kind="ExternalOutput").ap()

    with tile.TileContext(nc) as tc:
        with (
            tc.tile_pool(name="io", bufs=1) as io_pool,
            tc.tile_pool(name="small", bufs=1) as small,
            tc.tile_pool(name="epool", bufs=1) as epool,
        ):
            sel_sb = small.tile([128, SEL_COLS], fp16, tag="sel")
            nc.sync.dma_start(sel_sb[:], sel_d[:])
            gtd_sb = small.tile([128, F], fp16, tag="gtd")

            rad_t = [io_pool.tile([128, CHUNK_COLS], fp16, tag=f"rad{t}",
                                  name=f"rad{t}")
                     for t in range(N_CHUNK)]
            att_t = [io_pool.tile([128, CHUNK_COLS], fp16, tag=f"att{t}",
                                  name=f"att{t}")
                     for t in range(N_CHUNK)]

            # ---- all input DMAs issued up front; rad on the sync HWDGE
            # ring, att on the scalar ring. First chunk halved so matmuls
            # start after ~2 MiB; last chunk split to shrink the tail; gtd
            # queued last (needed only by phase 3).
            hm = CHUNK_COLS // 2
            nc.sync.dma_start(rad_t[0][:, 0:hm], rad_d[0, :, 0:hm])
            nc.scalar.dma_start(att_t[0][:, 0:hm], att_d[0, :, 0:hm])
            nc.sync.dma_start(rad_t[0][:, hm:], rad_d[0, :, hm:])
            nc.scalar.dma_start(att_t[0][:, hm:], att_d[0, :, hm:])
            for t in (1, 2):
                nc.sync.dma_start(rad_t[t][:], rad_d[t, :, :])
                nc.scalar.dma_start(att_t[t][:], att_d[t, :, :])
            lm = 6144
            nc.sync.dma_start(rad_t[3][:, 0:lm], rad_d[3, :, 0:lm])
            nc.scalar.dma_start(att_t[3][:, 0:lm], att_d[3, :, 0:lm])
            nc.sync.dma_start(rad_t[3][:, lm:], rad_d[3, :, lm:])
            nc.scalar.dma_start(att_t[3][:, lm:], att_d[3, :, lm:])
            nc.sync.dma_start(gtd_sb[:, 0:2048], gtd_d[:, 0:2048])
            nc.scalar.dma_start(gtd_sb[:, 2048:4096], gtd_d[:, 2048:4096])
            nc.sync.dma_start(gtd_sb[:, 4096:6144], gtd_d[:, 4096:6144])
            nc.scalar.dma_start(gtd_sb[:, 6144:8192], gtd_d[:, 6144:8192])

            # ---- main loop: s128 += rad_slice^T att_slice ----
            s16 = small.tile([128, 128], fp16, tag="s16")
            total = N_CHUNK * SLICES
            with tc.tile_pool(name="spsum", bufs=1, space="PSUM") as spsum:
                banks = [spsum.tile([128, 128], fp32, tag=f"s{b}",
                                    name=f"sbank{b}") for b in range(NB)]
                idx = 0
                for t in range(N_CHUNK):
                    for s in range(SLICES):
                        sl = slice(s * 128, (s + 1) * 128)
                        nc.tensor.matmul(
                            banks[idx % NB][:],
                            lhsT=rad_t[t][:, sl],
                            rhs=att_t[t][:, sl],
                            start=(idx < NB),
                            stop=(idx >= total - NB),
                        )
                        idx += 1

                acc = small.tile([128, 128], fp32, tag="acc")
                nc.vector.tensor_copy(acc[:], banks[0][:])
                for b in range(1, NB):
                    nc.vector.tensor_add(acc[:], acc[:], banks[b][:])
                nc.vector.tensor_copy(s16[:], acc[:])

            # ---- W build: 16 selection matmuls -> W_ps, scaled to fp16 ----
            wh = small.tile([64, 128], fp16, tag="wh")
            with tc.tile_pool(name="wpsum", bufs=1, space="PSUM") as wpsum:
                for C in range(4):
                    w_ps = wpsum.tile([64, 32], fp32, tag=f"w{C}", name=f"w{C}")
                    for g in range(4):
                        cs = (C * 4 + g) * 64
                        nc.tensor.matmul(w_ps[:],
                                         lhsT=sel_sb[:, cs:cs + 64],
                                         rhs=s16[:, g * 32:(g + 1) * 32],
                                         start=(g == 0), stop=(g == 3))
                    nc.vector.tensor_scalar_mul(
                        wh[:, C * 32:(C + 1) * 32], w_ps[:], float(SCALE))

                # PE warm-keepers bridging the epilogue gap
                warm = wpsum.tile([64, 64], fp32, tag="warm")
                for _ in range(6):
                    nc.tensor.matmul(warm[:], lhsT=sel_sb[:, 0:64],
                                     rhs=s16[:, 0:64], start=True, stop=True)

            # ---- phase 3: csi chunks over F ----
            csi_sb = small.tile([2, F], fp16, tag="csi")
            with (
                tc.tile_pool(name="tpsum", bufs=4, space="PSUM") as tpsum,
                tc.tile_pool(name="cpsum", bufs=2, space="PSUM") as cpsum,
            ):
                e_tiles = []
                for ci in range(N_FCHUNK):
                    fs = slice(ci * FCHUNK, (ci + 1) * FCHUNK)
                    t_ps = tpsum.tile([128, FCHUNK], fp32, tag="t",
                                      name=f"t{ci}")
                    nc.tensor.matmul(t_ps[:], lhsT=wh[:],
                                     rhs=gtd_sb[0:64, fs],
                                     start=True, stop=True)
                    e_sb = epool.tile([128, FCHUNK], fp16, tag=f"e{ci}",
                                      name=f"e{ci}")
                    nc.vector.tensor_mul(e_sb[:], gtd_sb[:, fs], t_ps[:])
                    e_tiles.append(e_sb)
                for ci in range(N_FCHUNK):
                    fs = slice(ci * FCHUNK, (ci + 1) * FCHUNK)
                    c_ps = cpsum.tile([2, FCHUNK], fp32, tag="c",
                                      name=f"c{ci}")
                    nc.tensor.matmul(c_ps[:], lhsT=sel_sb[:, 1024:1026],
                                     rhs=e_tiles[ci][:], start=True,
                                     stop=True)
                    if ci % 2:
                        nc.vector.tensor_copy(csi_sb[:, fs], c_ps[:])
                    else:
                        nc.scalar.copy(csi_sb[:, fs], c_ps[:])

            nc.sync.dma_start(out_d[:], csi_sb[:])

    nc.compile()
    return nc


def _pack_core(arr, core):
    """Core's complex64 shard -> (N_CHUNK, 128, CHUNK_COLS) fp16 with
    per-row [Re(32) | Im(32)] packing."""
    sh = arr[core * DIR_PER_CORE:(core + 1) * DIR_PER_CORE].reshape(-1, R)
    a = sh.real.astype(np.float16)
    b = sh.imag.astype(np.float16)
    rows = np.concatenate([a, b], axis=1)            # (65536, 64)
    return np.ascontiguousarray(rows.reshape(N_CHUNK, 128, CHUNK_COLS))


def _prep_gtd(fbv):
    """(128, F) f16: [Re.T(32); Im.T(32)] duplicated to 128 partitions."""
    fb = np.asarray(fbv)
    gbt = np.concatenate([fb.real.T, fb.imag.T], axis=0).astype(np.float16)
    return np.ascontiguousarray(np.concatenate([gbt, gbt], axis=0))


def _build_in_maps(attenuation_vectors, radiation_vectors,
                   frequency_basis_vectors):
    gtd = _prep_gtd(frequency_basis_vectors)
    sel = _build_sel()
    in_maps = []
    for c in range(N_CORES):
        in_maps.append({
            "rad": _pack_core(radiation_vectors, c),
            "att": _pack_core(attenuation_vectors, c),
            "gtd": gtd, "sel": sel,
        })
    return in_maps


def kernel(attenuation_vectors, radiation_vectors, frequency_basis_vectors):
    from concourse.bass_utils import run_bass_kernel_spmd

    if "nc" not in _NC_CACHE:
        _NC_CACHE["nc"] = build_nc()
    nc = _NC_CACHE["nc"]

    in_maps = _build_in_maps(attenuation_vectors, radiation_vectors,
                             frequency_basis_vectors)
    res = run_bass_kernel_spmd(nc, in_maps, core_ids=list(range(N_CORES)))
    acc = np.zeros((2, F), np.float64)
    for r in res.results:
        acc += r["csi"].astype(np.float64)
    return (acc[0] + 1j * acc[1]).astype(np.complex64)


# revision 5
# speedup vs baseline: 2.2263x; 1.0044x over previous
"""Trainium2 Bass kernel for LowRankRayTracer.

csi[f] = (delta_t/D) * v_f^T M v_f,  M = conj(rad)^T conj(att)  (R=32, complex)
contracted over N = D*K = 524288 rows.

Strategy (8 cores, memory-bound => minimize HBM bytes):
  - Tolerance is 2e-2; fp16-only inputs give ~6e-4 (validated in sim), so
    each f32 component is shipped as ONE fp16 (half the hi+lo baseline's
    traffic): 16 MiB/core of ray data + 2 MiB of frequency data.
  - Rows sharded 8 ways. Each core accumulates s128 = rad_pack^T att_pack
    (128x128, 2 rows/partition-slice) in 4 PSUM banks (banks 0/1 for the
    first half of the stream so their DVE fold overlaps the tail DMAs),
    then 16 tiny selection matmuls (+-1 consts) assemble W = [Wr|Wi]
    directly from s128 (host packs [Re|Im] per row so every extraction is
    32-aligned). Phase 3 per 1024-col chunk: T = W^T g (PE), e = g (.) T
    (DVE, half the chunks via an ACT fp16-copy so DVE runs in 2x mode),
    csi = ones^T e (PE, interleaved with T in issue order so the in-order
    PE queue pipelines), (2,1024) ACT copies out. Host sums partial csi.
  - All chunk tiles are statically resident in SBUF (20 MiB) so every DMA
    is issued up front and the input stream never stalls; first/last
    chunks are split so compute starts early and the post-DMA tail is
    short; gtd rides early so phase 3 never waits on it.
"""

import numpy as np

D, K, R = 4096, 128, 32
F = 8192
N_CORES = 8
DIR_PER_CORE = D // N_CORES              # 512
N_CHUNK = 4                              # macro chunks per tensor per core
CHUNK_COLS = 8192                        # fp16 per partition per chunk
SLICES = CHUNK_COLS // 128               # 64 matmul slices per chunk
SCALE = (200.0 / K) / D                  # delta_t / num_directions
FCHUNK = 1024
N_FCHUNK = F // FCHUNK                   # 8
SEL_COLS = 1152                          # 16*64 selection cols + ones2 + pad

_NC_CACHE = {}


def _build_sel():
    """(128, 1152) f16: 16 (128,64) +-1 selection matrices + ones2 cols.

    W 32-block (R,C) = sum of sigma*Q_uv;  Q_uv = S64[32u:+32, 32v:+32],
    S64 = s128[0:64,0:64] + s128[64:,64:].  Matmul g uses rhs cols
    [32g:32g+32] of s128; source (side, v) -> g = v + 2*side.
    """
    table = {
        (0, 0): [(+1, 0, 0), (-1, 1, 1)],   # Mr
        (1, 0): [(+1, 0, 1), (+1, 1, 0)],   # -Mi
        (0, 1): [(+1, 0, 1), (+1, 1, 0)],   # -Mi
        (1, 1): [(-1, 0, 0), (+1, 1, 1)],   # -Mr
        (0, 2): [(-1, 0, 1), (-1, 1, 0)],   # Mi
        (1, 2): [(+1, 0, 0), (-1, 1, 1)],   # Mr
        (0, 3): [(+1, 0, 0), (-1, 1, 1)],   # Mr
        (1, 3): [(+1, 0, 1), (+1, 1, 0)],   # -Mi
    }
    sel = np.zeros((128, SEL_COLS), np.float16)
    for (Rb, C), terms in table.items():
        for sigma, u, v in terms:
            for side in (0, 1):
                g = v + 2 * side
                for r0 in range(32):
                    p = 64 * side + 32 * u + r0
                    sel[p, (C * 4 + g) * 64 + 32 * Rb + r0] = sigma
    sel[0:64, 1024] = 1.0
    sel[64:128, 1025] = 1.0
    return sel


def build_nc():
    import concourse.bacc as bacc
    import concourse.mybir as mybir
    import concourse.tile as tile

    fp32 = mybir.dt.float32
    fp16 = mybir.dt.float16
    nc = bacc.Bacc(trn_type="TRN2", target_bir_lowering=False, debug=False)

    rad_d = nc.dram_tensor("rad", [N_CHUNK, 128, CHUNK_COLS], fp16,
                           kind="ExternalInput").ap()
    att_d = nc.dram_tensor("att", [N_CHUNK, 128, CHUNK_COLS], fp16,
                           kind="ExternalInput").ap()
    gtd_d = nc.dram_tensor("gtd", [128, F], fp16, kind="ExternalInput").ap()
    sel_d = nc.dram_tensor("sel", [128, SEL_COLS], fp16,
                           kind="ExternalInput").ap()
    out_d = nc.dram_tensor("csi", [2, F], fp16, kind="ExternalOutput").ap()

    with tile.TileContext(nc) as tc:
        with (
            tc.tile_pool(name="io", bufs=1) as io_pool,
            tc.tile_pool(name="small", bufs=1) as small,
            tc.tile_pool(name="epool", bufs=1) as epool,
        ):
            sel_sb = small.tile([128, SEL_COLS], fp16, tag="sel")
            gtd_sb = small.tile([128, F], fp16, tag="gtd")

            rad_t = [io_pool.tile([128, CHUNK_COLS], fp16, tag=f"rad{t}",
                                  name=f"rad{t}")
                     for t in range(N_CHUNK)]
            att_t = [io_pool.tile([128, CHUNK_COLS], fp16, tag=f"att{t}",
                                  name=f"att{t}")
                     for t in range(N_CHUNK)]

            # ---- all input DMAs issued up front; rad on the sync HWDGE
            # ring, att on the scalar ring. First chunk split so matmuls
            # start after ~2 MiB; gtd/sel ride early (phase 3 and the
            # epilogue never wait on them); last chunk split so the
            # post-DMA matmul tail is one 0.5 MiB piece.
            hm = CHUNK_COLS // 2
            nc.sync.dma_start(rad_t[0][:, 0:hm], rad_d[0, :, 0:hm])
            nc.scalar.dma_start(att_t[0][:, 0:hm], att_d[0, :, 0:hm])
            nc.sync.dma_start(rad_t[0][:, hm:], rad_d[0, :, hm:])
            nc.scalar.dma_start(att_t[0][:, hm:], att_d[0, :, hm:])
            nc.sync.dma_start(gtd_sb[:, 0:4096], gtd_d[:, 0:4096])
            nc.scalar.dma_start(gtd_sb[:, 4096:8192], gtd_d[:, 4096:8192])
            nc.sync.dma_start(sel_sb[:], sel_d[:])
            for t in (1, 2):
                nc.sync.dma_start(rad_t[t][:], rad_d[t, :, :])
                nc.scalar.dma_start(att_t[t][:], att_d[t, :, :])
            lm = 6144
            nc.sync.dma_start(rad_t[3][:, 0:lm], rad_d[3, :, 0:lm])
            nc.scalar.dma_start(att_t[3][:, 0:lm], att_d[3, :, 0:lm])
            nc.sync.dma_start(rad_t[3][:, lm:], rad_d[3, :, lm:])
            nc.scalar.dma_start(att_t[3][:, lm:], att_d[3, :, lm:])

            # ---- main loop: s128 += rad_slice^T att_slice ----
            # chunks 0-1 accumulate in banks 0/1, chunks 2-3 in banks 2/3:
            # the 0/1 fold runs on DVE while the tail DMAs stream.
            s16 = small.tile([128, 128], fp16, tag="s16")
            acc01 = small.tile([128, 128], fp32, tag="acc01")
            acc = small.tile([128, 128], fp32, tag="acc")
            with tc.tile_pool(name="spsum", bufs=1, space="PSUM") as spsum:
                banks = [spsum.tile([128, 128], fp32, tag=f"s{b}",
                                    name=f"sbank{b}") for b in range(4)]
                for t in range(N_CHUNK):
                    b0 = (t // 2) * 2
                    for s in range(SLICES):
                        sl = slice(s * 128, (s + 1) * 128)
                        nc.tensor.matmul(
                            banks[b0 + s % 2][:],
                            lhsT=rad_t[t][:, sl],
                            rhs=att_t[t][:, sl],
                            start=(t % 2 == 0 and s < 2),
                            stop=(t % 2 == 1 and s >= SLICES - 2),
                        )
                    if t == 1:
                        nc.vector.tensor_copy(acc01[:], banks[0][:])
                        nc.vector.tensor_add(acc01[:], acc01[:], banks[1][:])
                acc23 = small.tile([128, 128], fp32, tag="acc23")
                nc.vector.tensor_copy(acc23[:], banks[2][:])
                nc.vector.tensor_add(acc23[:], acc23[:], banks[3][:])
                nc.vector.tensor_add(acc[:], acc01[:], acc23[:])
                nc.vector.tensor_copy(s16[:], acc[:])

            # ---- W build: 16 selection matmuls -> W_ps, scaled to fp16 ----
            wh = small.tile([64, 128], fp16, tag="wh")
            with tc.tile_pool(name="wpsum", bufs=1, space="PSUM") as wpsum:
                for C in range(4):
                    w_ps = wpsum.tile([64, 32], fp32, tag=f"w{C}",
                                      name=f"w{C}")
                    for g in range(4):
                        cs = (C * 4 + g) * 64
                        nc.tensor.matmul(w_ps[:],
                                         lhsT=sel_sb[:, cs:cs + 64],
                                         rhs=s16[:, g * 32:(g + 1) * 32],
                                         start=(g == 0), stop=(g == 3))
                    nc.vector.tensor_scalar_mul(
                        wh[:, C * 32:(C + 1) * 32], w_ps[:], float(SCALE))

            # ---- phase 3: csi chunks over F (8 x 1024 cols) ----
            # PE issue order interleaves T and ones matmuls with a
            # one-chunk lag; even chunks route T through an ACT fp16 copy
            # so the DVE multiply runs in 2x mode; (2,1024) csi copies on
            # ACT; output DMA split in two so the first half overlaps.
            csi_sb = small.tile([2, F], fp16, tag="csi")
            with (
                tc.tile_pool(name="tpsum", bufs=2, space="PSUM") as tpsum,
                tc.tile_pool(name="cpsum", bufs=2, space="PSUM") as cpsum,
            ):
                t_tiles = [None] * N_FCHUNK
                e_tiles = [None] * N_FCHUNK
                c_tiles = [None] * N_FCHUNK

                def issue_t(ci):
                    fs = slice(ci * FCHUNK, (ci + 1) * FCHUNK)
                    t_ps = tpsum.tile([128, FCHUNK], fp32, tag="t",
                                      name=f"t{ci}")
                    for k in (0, 1):
                        ks = slice(k * 512, (k + 1) * 512)
                        gs = slice(ci * FCHUNK + k * 512,
                                   ci * FCHUNK + (k + 1) * 512)
                        nc.tensor.matmul(t_ps[:, ks], lhsT=wh[:],
                                         rhs=gtd_sb[0:64, gs],
                                         start=True, stop=True)
                    e_sb = epool.tile([128, FCHUNK], fp16, tag=f"e{ci}",
                                      name=f"e{ci}")
                    if ci % 2 == 0:
                        tc_sb = epool.tile([128, FCHUNK], fp16,
                                           tag=f"tc{ci}", name=f"tc{ci}")
                        nc.scalar.copy(tc_sb[:], t_ps[:])
                        nc.vector.tensor_mul(e_sb[:], gtd_sb[:, fs], tc_sb[:])
                    else:
                        nc.vector.tensor_mul(e_sb[:], gtd_sb[:, fs], t_ps[:])
                    t_tiles[ci] = t_ps
                    e_tiles[ci] = e_sb

                def issue_ones(ci):
                    fs = slice(ci * FCHUNK, (ci + 1) * FCHUNK)
                    c_ps = cpsum.tile([2, FCHUNK], fp32, tag="c",
                                      name=f"c{ci}")
                    for k in (0, 1):
                        ks = slice(k * 512, (k + 1) * 512)
                        nc.tensor.matmul(c_ps[:, ks],
                                         lhsT=sel_sb[:, 1024:1026],
                                         rhs=e_tiles[ci][:, ks],
                                         start=True, stop=True)
                    nc.scalar.copy(csi_sb[:, fs], c_ps[:])
                    c_tiles[ci] = c_ps

                issue_t(0)
                for ci in range(1, N_FCHUNK):
                    issue_t(ci)
                    issue_ones(ci - 1)
                issue_ones(N_FCHUNK - 1)

            nc.sync.dma_start(out_d[:, 0:4096], csi_sb[:, 0:4096])
            nc.scalar.dma_start(out_d[:, 4096:8192], csi_sb[:, 4096:8192])

    nc.compile()
    return nc


def _pack_core(arr, core):
    """Core's complex64 shard -> (N_CHUNK, 128, CHUNK_COLS) fp16 with
    per-row [Re(32) | Im(32)] packing."""
    sh = arr[core * DIR_PER_CORE:(core + 1) * DIR_PER_CORE].reshape(-1, R)
    a = sh.real.astype(np.float16)
    b = sh.imag.astype(np.float16)
    rows = np.concatenate([a, b], axis=1)            # (65536, 64)
    return np.ascontiguousarray(rows.reshape(N_CHUNK, 128, CHUNK_COLS))


def _prep_gtd(fbv):
    """(128, F) f16: [Re.T(32); Im.T(32)] duplicated to 128 partitions."""
    fb = np.asarray(fbv)
    gbt = np.concatenate([fb.real.T, fb.imag.T], axis=0).astype(np.float16)
    return np.ascontiguousarray(np.concatenate([gbt, gbt], axis=0))


def _build_in_maps(attenuation_vectors, radiation_vectors,
                   frequency_basis_vectors):
    gtd = _prep_gtd(frequency_basis_vectors)
    sel = _build_sel()
    in_maps = []
    for c in range(N_CORES):
        in_maps.append({
            "rad": _pack_core(radiation_vectors, c),
            "att": _pack_core(attenuation_vectors, c),
            "gtd": gtd, "sel": sel,
        })
    return in_maps


def kernel(attenuation_vectors, radiation_vectors, frequency_basis_vectors):
    from concourse.bass_utils import run_bass_kernel_spmd

    if "nc" not in _NC_CACHE:
        _NC_CACHE["nc"] = build_nc()
    nc = _NC_CACHE["nc"]

    in_maps = _build_in_maps(attenuation_vectors, radiation_vectors,
                             frequency_basis_vectors)
    res = run_bass_kernel_spmd(nc, in_maps, core_ids=list(range(N_CORES)))
    acc = np.zeros((2, F), np.float64)
    for r in res.results:
        acc += r["csi"].astype(np.float64)
    return (acc[0] + 1j * acc[1]).astype(np.complex64)
